# revision 28
# baseline (speedup 1.0000x reference)
"""DeformableConv2d Trainium2 kernel (V2).

Data-parallel over batch: 8 samples -> 8 NeuronCores, one sample per core.

Key identity: bilinear sampling is linear in the image, so it commutes with
the channel contraction:
    out[o,q] = sum_k bilinear(G_k, py_k(q), px_k(q))[o] + bias[o]
with G_k = W_k @ x (a 1x1 conv per tap); the (ki,kj) tap displacement is
folded into the gather position.

V2 layout: the DRAM workspace holds 256-B *y-pair records*
    rec_k(y, x) = [G_k(y,x) 64ch, G_k(y+1,x) 64ch]   (bf16)
on a 120x120 zero-padded grid (GPAD=4 absorbs offsets; zero pad == OOB-zero
semantics). One dma_gather element (elem_size 512B, elem_step 256B) at
idx = y0*120+x0 fetches records (y0,x0),(y0,x0+1) = ALL FOUR bilinear
corners [g00,g10,g01,g11] -- half the descriptors/instructions of the
2-gather x-pair scheme. The y-pair comes from a second, row-shifted PE
matmul (PE has headroom), not extra copies.

Combine: per (tap, half): acc += sum_c w_c * g_c with the 4 corner weights
precomputed per pixel and stored as *duplicated bf16 pairs* [w,w] so the
weight operand's innermost AP dim is step-1/count-2 -> DVE 2x_1P mode for
the big multiply (a 0-stride innermost broadcast would force 1x).

Device pipeline per core:
  A) offset conv on PE (bf16, 9 accumulating taps over padded x).
  B) index math on DVE: floor/frac; idx = y0*120+x0 (int16), replicated to
     128 partitions for the gather's 16-wrapped SWDGE layout; w4dup pair
     weights.
  C) G staging in 3 tap-groups of 3 (so gathers of group g overlap staging
     of group g+1): per grid column, two row-shifted matmuls into a
     bank-aligned [113, 2, 512] PSUM tile; one ScalarE drain interleaves
     [shift,k,o] into SBUF; per (28-column group, tap) SWDGE DMAs write
     y-pair records (small concurrent DMAs overlap on the rings, and
     per-tap granularity lets gather(k) wait only on tap k's slices);
     3 bulk DMAs zero-fill the pad bands.
  D) dma_gather (GPSIMD SWDGE) DRAM -> SBUF, 512-B elements, pixel-major
     output [q%128, q//128, 256]. Each (tap, half) is split into 2 chunk
     gathers so a tap's 4 chunks land on all 4 SWDGE queues -> all 4 Q7
     core pairs run descriptor generation concurrently (desc-gen is the
     dominant serial cost of dma_gather).
  E) DVE: one 2x-mode multiply by w4dup + 4 accumulate-adds per (tap,half).
"""

import numpy as np

K = 3
B, CIN, COUT, H, W = 8, 64, 64, 112, 112
HW = H * W                 # 12544
NPAD = H + 2               # 114 (conv pad=1)
GPAD = 4                   # padding of the gather grid
GP = H + 2 * GPAD          # 120
GROWS = GP * GP            # 14400 records in the gather grid
NQB = HW // 128            # 98 q-blocks of 128 pixels
NPLANE = 2 * K * K         # 18 offset planes (9 dy then 9 dx)
HALF = HW // 2             # 6272 pixels per gather half
HSLOT = HALF // 128        # 49
XG = 28                    # grid columns per staging DMA group
NXG = H // XG              # 4 groups (112 staged columns)
YR = H + 1                 # 113 staged record rows (y = 3..115)
REC = 2 * COUT             # 128 elems = 256 B per record

_CACHE = {}


def _build_program(weights, debug_skip=()):
    import ml_dtypes
    import concourse.bass as bass
    import concourse.bacc as bacc
    import concourse.mybir as mybir
    from concourse.tile import TileContext

    dt = mybir.dt
    ALU = mybir.AluOpType
    AP = bass.AP
    bf = ml_dtypes.bfloat16

    nc = bacc.Bacc("TRN2", target_bir_lowering=False, debug=False,
                   num_swdge_queues=4)

    offset_w = weights["offset_w"]  # [18, 64, 3, 3]
    offset_b = weights["offset_b"]  # [18]
    deform_w = weights["deform_w"]  # [64, 64, 3, 3]
    deform_b = weights["deform_b"]  # [64]

    # ---- host-side constants (baked into the NEFF) ----
    # offset conv lhsT per tap: [64, 18]; output plane j<9 = dy of tap j,
    # j>=9 = dx of tap j-9.
    woff = np.zeros((9, CIN, NPLANE), np.float32)
    for k in range(9):
        ki, kj = k // 3, k % 3
        for j in range(9):
            woff[k, :, j] = offset_w[2 * j, :, ki, kj]
            woff[k, :, 9 + j] = offset_w[2 * j + 1, :, ki, kj]

    # G conv rhs: wg[c, k*64+o] = deform_w[o, c, ki, kj]
    wg = np.zeros((CIN, 9 * COUT), np.float32)
    for k in range(9):
        ki, kj = k // 3, k % 3
        wg[:, k * 64:(k + 1) * 64] = deform_w[:, :, ki, kj].T

    # base grid [128, 18, 98] fp32 (+GPAD baked in, offset_b folded in)
    qs = np.arange(HW)
    ho, wo = qs // W, qs % W
    base = np.zeros((NPLANE, HW), np.float32)
    for k in range(9):
        ki, kj = k // 3, k % 3
        base[k] = ho + ki - 1 + GPAD + float(offset_b[2 * k])
        base[9 + k] = wo + kj - 1 + GPAD + float(offset_b[2 * k + 1])
    base_re = np.ascontiguousarray(
        base.reshape(NPLANE, NQB, 128).transpose(2, 0, 1))  # [128, 18, 98]

    bias_rep = np.broadcast_to(
        deform_b.astype(np.float32)[None, :], (128, COUT))

    woff_c = nc.inline_tensor(
        np.ascontiguousarray(woff.transpose(1, 0, 2)).reshape(
            CIN, 9 * NPLANE).astype(bf), name="woffc")
    wg_c = nc.inline_tensor(wg.astype(bf), name="wgc")
    base_c = nc.inline_tensor(
        base_re.reshape(128, NPLANE * NQB), name="basec")
    bias_c = nc.inline_tensor(np.ascontiguousarray(bias_rep).astype(bf),
                              name="biasc")
    ident_c = nc.inline_tensor(np.eye(NPLANE, dtype=np.float32).astype(bf),
                               name="ident")
    sel = np.zeros((8, 128, 16), np.float32)
    for s in range(8):
        for t in range(16):
            sel[s, 16 * s + t, t] = 1.0
    sel_c = nc.inline_tensor(
        np.ascontiguousarray(sel.transpose(1, 0, 2)).reshape(
            128, 8 * 16).astype(bf), name="selc")

    xpad = nc.dram_tensor("xpad", [CIN, NPAD * NPAD], dt.bfloat16,
                          kind="ExternalInput")
    out_t = nc.dram_tensor("out_t", [HW, COUT], dt.bfloat16,
                           kind="ExternalOutput")
    # +1 record per tap so a gather element at the last record stays within
    # the tap plane (idx 14399 reads records 14399,14400).
    gws = nc.dram_tensor("gws", [9, GROWS + 1, REC], dt.bfloat16,
                         kind="Internal")
    TAP = (GROWS + 1) * REC  # elem stride between tap planes

    with TileContext(nc) as tc:
        with (
            tc.tile_pool(name="const", bufs=1) as constp,
            tc.tile_pool(name="xsb", bufs=1) as xsbp,
            tc.tile_pool(name="persist", bufs=1) as perp,
            tc.tile_pool(name="accp", bufs=1) as accp,
            tc.tile_pool(name="zz", bufs=1) as zzp,
        ):
            # ---- constants + input ----
            woff_sb = constp.tile([CIN, 9, NPLANE], dt.bfloat16)
            nc.sync.dma_start(
                out=woff_sb[:],
                in_=woff_c[:].rearrange("c (k m) -> c k m", k=9))
            wg_sb = constp.tile([CIN, 9 * COUT], dt.bfloat16)
            nc.sync.dma_start(out=wg_sb[:], in_=wg_c[:])
            base_sb = constp.tile([128, NPLANE, NQB], dt.float32)
            nc.sync.dma_start(
                out=base_sb[:],
                in_=base_c[:].rearrange("p (a b) -> p a b", a=NPLANE))
            bias_sb = constp.tile([128, COUT], dt.bfloat16)
            nc.sync.dma_start(out=bias_sb[:], in_=bias_c[:])
            ident_sb = constp.tile([NPLANE, NPLANE], dt.bfloat16)
            nc.sync.dma_start(out=ident_sb[:], in_=ident_c[:])
            sel_sb = constp.tile([128, 8, 16], dt.bfloat16)
            nc.sync.dma_start(
                out=sel_sb[:], in_=sel_c[:].rearrange("p (s t) -> p s t", s=8))

            x_sb = xsbp.tile([CIN, NPAD, NPAD], dt.bfloat16)
            nc.sync.dma_start(
                out=x_sb[:], in_=xpad[:].rearrange("c (a b) -> c a b", a=NPAD))

            # ---- zero-fill of the record-grid pad bands ----
            # Content lives at record rows 3..115, cols 4..115. Zero bands
            # (record units, row-major y*120+x):
            #   top:    records 0..363        (rows 0-2 + row 3 cols 0-3)
            #   mid:    per row y in 3..114: the 8 contiguous records
            #           (y,116)..(y+1,3) straddling the row boundary
            #   bottom: records 13916..14400  ((115,116) .. the pad record)
            zz = zzp.tile([128, 1024], dt.bfloat16)
            nc.vector.memset(zz[:], 0)
            if "gstage" not in debug_skip:
                zzap = zz[:]
                ZSP = zzap.ap[0][0]
                for k in range(9):
                    gk = gws[k, :, :]
                    dst = AP(gk.tensor, gk.offset,
                             [[364, 128], [1, 364]])
                    src = AP(zzap.tensor, zzap.offset,
                             [[ZSP, 128], [1, 364]])
                    nc.sync.dma_start(out=dst, in_=src)
                    dst = AP(gk.tensor, gk.offset + (3 * GP + 116) * REC,
                             [[GP * REC, 112], [1, 1024]])
                    src = AP(zzap.tensor, zzap.offset,
                             [[ZSP, 112], [1, 1024]])
                    nc.sync.dma_start(out=dst, in_=src)
                    dst = AP(gk.tensor, gk.offset + (115 * GP + 116) * REC,
                             [[485, 128], [1, 485]])
                    src = AP(zzap.tensor, zzap.offset,
                             [[ZSP, 128], [1, 485]])
                    nc.sync.dma_start(out=dst, in_=src)

            # ---- A) offset conv + B) index math (scoped pools) ----
            ab = tc.tile_pool(name="idxm", bufs=1)
            idxp = ab.__enter__()
            psA_cm = tc.tile_pool(name="psA", bufs=2, space="PSUM")
            psA = psA_cm.__enter__()
            psT_cm = tc.tile_pool(name="psT", bufs=1, space="PSUM", side="right")
            psT = psT_cm.__enter__()
            off_sb = idxp.tile([NPLANE, HW], dt.bfloat16)
            RB = 4
            for blk in range(H // RB):
                ps = psA.tile([NPLANE, RB * W], dt.float32)
                for k in range(9):
                    ki, kj = k // 3, k % 3
                    rhs = x_sb[:, blk * RB + ki: blk * RB + ki + RB,
                               kj: kj + W]
                    nc.tensor.matmul(ps[:], woff_sb[:, k, :], rhs,
                                     start=(k == 0), stop=(k == 8))
                nc.scalar.copy(
                    out=off_sb[:, blk * RB * W:(blk + 1) * RB * W], in_=ps[:])
            psA_cm.__exit__(None, None, None)

            # rearrange [18, HW] -> [128, 18, 98] via PE transposes,
            # 8 transposes packed per PSUM tile -> 1 drain per 8 blocks.
            off_re = idxp.tile([128, NPLANE, NQB], dt.float32)
            qb = 0
            while qb < NQB:
                nn = min(8, NQB - qb)
                pst = psT.tile([128, 8, 32], dt.bfloat16, tag="pstT")
                for j in range(nn):
                    nc.tensor.transpose(
                        pst[:, j, 0:NPLANE],
                        off_sb[:, (qb + j) * 128:(qb + j + 1) * 128],
                        ident_sb[:])
                orap = off_re[:]
                OSP = orap.ap[0][0]
                prap = pst[:]
                PSP = prap.ap[0][0]
                dstr = AP(orap.tensor, orap.offset + qb,
                          [[OSP, 128], [NQB, NPLANE], [1, nn]])
                srcr = AP(prap.tensor, prap.offset,
                          [[PSP, 128], [1, NPLANE], [32, nn]])
                nc.scalar.copy(out=dstr, in_=srcr)
                qb += nn

            # ---- B) index math ----
            pyx = idxp.tile([128, NPLANE, NQB], dt.float32)
            nc.vector.tensor_add(out=pyx[:], in0=off_re[:], in1=base_sb[:])
            # floor(pyx) for pyx>=0: round-to-int via +-2^23, then
            # subtract 1 where rounding went up.
            M23 = 8388608.0
            rnd = idxp.tile([128, NPLANE, NQB], dt.float32)
            nc.vector.tensor_scalar(rnd[:], pyx[:], M23, M23, ALU.add,
                                    ALU.subtract)
            dgt = idxp.tile([128, NPLANE, NQB], dt.float32)
            nc.vector.tensor_tensor(out=dgt[:], in0=rnd[:], in1=pyx[:],
                                    op=ALU.is_gt)
            flr = rnd
            nc.vector.tensor_sub(out=flr[:], in0=rnd[:], in1=dgt[:])
            frac = dgt
            nc.vector.tensor_sub(out=frac[:], in0=pyx[:], in1=flr[:])
            # fractional weights as duplicated bf16 pairs [f, f] so the
            # combine multiply keeps DVE 2x_1P (innermost step-1 pair).
            fdup = idxp.tile([128, NPLANE, NQB, 2], dt.bfloat16)
            nc.vector.tensor_copy(
                out=fdup[:],
                in_=frac[:, :, :, None].to_broadcast((128, NPLANE, NQB, 2)))
            gdup = idxp.tile([128, NPLANE, NQB, 2], dt.bfloat16)
            nc.vector.tensor_scalar(gdup[:], fdup[:], -1.0, 1.0, ALU.mult,
                                    ALU.add)
            # 4 corner weights per (tap, pixel), pair-duplicated; corner
            # order matches the gathered element [g00, g10, g01, g11]:
            #   c0=wy0*wx0  c1=wy1*wx0  c2=wy0*wx1  c3=wy1*wx1
            w4dup = perp.tile([128, 9, 4, NQB, 2], dt.bfloat16)
            for c, (yt, xt) in enumerate(((0, 0), (1, 0), (0, 1), (1, 1))):
                wy = (gdup, fdup)[yt]
                wx = (gdup, fdup)[xt]
                nc.vector.tensor_mul(out=w4dup[:, :, c],
                                     in0=wy[:, 0:9], in1=wx[:, 9:18])
            abq = tc.tile_pool(name="idxb", bufs=1, side="right")
            idxq = abq.__enter__()
            flr_bf = idxq.tile([128, NPLANE, NQB], dt.bfloat16)
            nc.vector.tensor_copy(out=flr_bf[:], in_=flr[:])
            ab.__exit__(None, None, None)

            # ---- C) G staging pools + group-0 staging, hoisted ahead of
            # the transpose/index phase: staging only needs x and wg, so
            # its PE matmuls follow the offset conv immediately and the
            # ACT drains + record DMAs run under the index pipeline,
            # getting tap 0-2 records ready ~50us sooner.
            psG_cm = tc.tile_pool(name="psG", bufs=3, space="PSUM")
            psG = psG_cm.__enter__()
            gsb_cm = tc.tile_pool(name="gsb", bufs=2)
            gsbp = gsb_cm.__enter__()

            def stage_group(kl, kh):
                # stage y-pair records for taps kl..kh-1 (all columns).
                NK = kh - kl
                for xg in range(NXG):
                    # k-major so the record DMA's inner run (per y,k) is
                    # the contiguous XG*REC block
                    gsb = gsbp.tile([YR, 3, XG, 2, COUT], dt.bfloat16)
                    for s in range(XG):
                        gx = 4 + xg * XG + s
                        psa = psG.tile([YR, 2, 512], dt.float32,
                                       tag="psa")
                        for sh in range(2):
                            lhsT = x_sb[:, sh:sh + YR, gx - 3]
                            nc.tensor.matmul(
                                psa[:, sh, 0:NK * COUT], lhsT,
                                wg_sb[:, kl * COUT:kh * COUT],
                                start=True, stop=True)
                        # drain [y, sh, k, o] -> gsb[y, k, s, sh, o]
                        ga = gsb[:]
                        GS = ga.ap[0][0]
                        pa = psa[:]
                        PS = pa.ap[0][0]
                        dsta = AP(ga.tensor,
                                  ga.offset + s * (2 * COUT),
                                  [[GS, YR], [COUT, 2],
                                   [XG * 2 * COUT, NK], [1, COUT]])
                        srca = AP(pa.tensor, pa.offset,
                                  [[PS, YR], [512, 2], [COUT, NK],
                                   [1, COUT]])
                        nc.scalar.copy(out=dsta, in_=srca)
                    # per-(xg, tap, y-half) record DMAs: one SWDGE DMA of
                    # this shape is serviced by ~one SDMA engine (~27 GB/s
                    # = 7KB descriptors at ~265ns each); aggregate rate
                    # scales with the number of DMAs in flight, so split
                    # finely. Per-tap granularity also lets gather(k) wait
                    # only on tap k's slices instead of the whole group.
                    ga = gsb[:]
                    GS = ga.ap[0][0]
                    for k in range(kl, kh):
                        gk = gws[k, :, :]
                        for y0, y1 in ((0, 57), (57, YR)):
                            dst = AP(gk.tensor,
                                     gk.offset
                                     + ((3 + y0) * GP + 4 + xg * XG) * REC,
                                     [[GP * REC, y1 - y0], [1, XG * REC]])
                            src = AP(ga.tensor,
                                     ga.offset + y0 * GS
                                     + (k - kl) * (XG * 2 * COUT),
                                     [[GS, y1 - y0], [1, XG * 2 * COUT]])
                            nc.gpsimd.dma_start(out=dst, in_=src)

            if "gstage" not in debug_skip:
                stage_group(0, 3)


            # ---- idx selection path first (PE/ScalarE ahead of staging) --
            # floor values are integers <= ~121 -> exact in bf16; fold
            # partition axis q_lo=(16s+t) into [16(t), ..., 8(s)] via 8
            # selection matmuls so the gather's 16-wrapped index layout is
            # reachable without per-element DMA. flrs is s-major so each
            # PSUM drain is contiguous; the idx combine below reorders to
            # the s-minor layout the gather consumes via its APs.
            flrs = idxq.tile([16, 8, NPLANE, 2, HSLOT], dt.bfloat16)
            for s in range(8):
                for (pl0, PL) in ((0, 5), (5, 4), (9, 5), (14, 4)):
                    pst2 = psT.tile([16, 5 * NQB], dt.float32, tag="pst2")
                    nc.tensor.matmul(pst2[:, 0:PL * NQB], sel_sb[:, s, :],
                                     flr_bf[:, pl0:pl0 + PL, :],
                                     start=True, stop=True)
                    nc.scalar.copy(
                        out=flrs[:, s, pl0:pl0 + PL, :, :],
                        in_=pst2[:, 0:PL * NQB].rearrange(
                            "p (a h q) -> p a h q", a=PL, h=2))
            # idx = yfloor*120 + xfloor (fp32 internal -> exact int16),
            # with the s-major -> s-minor reorder folded into the APs.
            idx_w16 = idxq.tile([16, 9, 2, HSLOT, 8], dt.int16)
            fap = flrs[:]
            FSP = fap.ap[0][0]
            iap = idx_w16[:]
            ISP2 = iap.ap[0][0]
            SSTRIDE = NPLANE * 2 * HSLOT
            for s in range(8):
                in_y = AP(fap.tensor, fap.offset + s * SSTRIDE,
                          [[FSP, 16], [2 * HSLOT, 9], [1, 2 * HSLOT]])
                in_x = AP(fap.tensor, fap.offset + s * SSTRIDE
                          + 9 * 2 * HSLOT,
                          [[FSP, 16], [2 * HSLOT, 9], [1, 2 * HSLOT]])
                out_i = AP(iap.tensor, iap.offset + s,
                           [[ISP2, 16], [2 * HSLOT * 8, 9], [8, 2 * HSLOT]])
                nc.vector.scalar_tensor_tensor(
                    out=out_i, in0=in_y, scalar=float(GP),
                    in1=in_x, op0=ALU.mult, op1=ALU.add)
            # replicate partitions 0:16 -> all 128 (idx_w persists)
            idx_w = perp.tile([128, 9, 2, 8 * HSLOT], dt.int16)
            i16 = idx_w16[:]
            wap = idx_w[:]
            WSPAN = wap.ap[0][0]
            ISPAN = i16.ap[0][0]
            rep_src = AP(i16.tensor, i16.offset,
                         [[ISPAN, 16], [1, ISPAN]])
            for g in range(8):
                rep_dst = AP(wap.tensor, wap.offset + 16 * g * WSPAN,
                             [[WSPAN, 16], [1, WSPAN]])
                nc.gpsimd.dma_start(out=rep_dst, in_=rep_src)

            psT_cm.__exit__(None, None, None)
            abq.__exit__(None, None, None)

            # ---- D/E) gather + combine, tap-pipelined ----
            gat_cm = tc.tile_pool(name="gat", bufs=2)
            gatp = gat_cm.__enter__()
            acc = accp.tile([128, NQB, COUT], dt.bfloat16)
            nc.vector.tensor_copy(
                out=acc[:],
                in_=bias_sb[:, None, :].to_broadcast((128, NQB, COUT)))

            def gather_combine(k):
                for h in range(2):
                    gg = gatp.tile([128, HSLOT, 4 * COUT], dt.bfloat16,
                                   tag="gg")
                    gk = gws[k, :, :]
                    src0 = AP(gk.tensor, gk.offset,
                              [[REC, GROWS], [1, 2 * REC]])
                    # split each half across 2 SWDGE queues (so each tap's
                    # 4 chunks land on all 4 queues = all 4 Q7 core pairs
                    # desc-gen concurrently). Chunk boundary at a slot
                    # multiple (f multiple of 8) keeps out slots aligned.
                    for c, (s0, s1) in enumerate(((0, 25), (25, HSLOT))):
                        nidx = (s1 - s0) * 128
                        idxs = idx_w[:, k, h, s0 * 8:s1 * 8]
                        nc.gpsimd.dma_gather(
                            out_ap=gg[:, s0:s1, :], in_ap=src0, idxs_ap=idxs,
                            num_idxs=nidx, num_idxs_reg=nidx,
                            elem_size=2 * REC, elem_step=REC,
                            single_packet=False, queue_num=(2 * h + c) % 4)

                    # per-corner m_c = g_c * w_c (in-place, 2x_1P via the
                    # step-1 pair weights; ISA caps TT at 3 free dims so
                    # one op per corner), then acc += m_c.
                    wt = w4dup[:]
                    WSP = wt.ap[0][0]
                    ggv = gg[:].rearrange("p s (c u d) -> p s c u d",
                                          c=4, d=2)
                    a = acc[:, h * HSLOT:(h + 1) * HSLOT, :]
                    gc = gg[:].rearrange("p s (c o) -> p s c o", c=4)
                    for c in range(4):
                        woff0 = (wt.offset + k * (4 * NQB * 2)
                                 + c * (NQB * 2) + h * (HSLOT * 2))
                        win = AP(wt.tensor, woff0,
                                 [[WSP, 128], [2, HSLOT],
                                  [0, COUT // 2], [1, 2]])
                        gcv = ggv[:, :, c]
                        nc.vector.tensor_mul(out=gcv, in0=gcv, in1=win)
                        nc.vector.tensor_add(out=a, in0=a,
                                             in1=gc[:, :, c, :])

            GROUPS = ((0, 3), (3, 6), (6, 9))
            if "gstage" not in debug_skip:
                for gi, (kl, kh) in enumerate(GROUPS):
                    # group 0 was staged early (above); issue group g+1's
                    # staging ahead of group g's gathers so PE/ACT/record
                    # DMAs run one group ahead of the gather pipeline.
                    if gi + 1 < len(GROUPS):
                        stage_group(*GROUPS[gi + 1])
                    if "gather" not in debug_skip:
                        for k in range(kl, kh):
                            gather_combine(k)

            gat_cm.__exit__(None, None, None)
            gsb_cm.__exit__(None, None, None)
            psG_cm.__exit__(None, None, None)

            # out: [128, 98, 64] -> DRAM [12544, 64] with q = qblk*128+q_lo
            oap2 = out_t[:]
            dst = AP(oap2.tensor, 0, [[COUT, 128], [128 * COUT, NQB],
                                      [1, COUT]])
            nc.sync.dma_start(out=dst, in_=acc[:])

    nc.compile()
    return nc


def _host_prep(x):
    """Per-core input prep: pad=1 + bf16 + channel-major."""
    import ml_dtypes
    xp = np.zeros((CIN, NPAD, NPAD), np.float32)
    xp[:, 1:113, 1:113] = x
    return np.ascontiguousarray(
        xp.reshape(CIN, NPAD * NPAD)).astype(ml_dtypes.bfloat16)


def build(x, offset_w, offset_b, deform_w, deform_b, debug_skip=()):
    """Build the Bass program + per-core input maps (shared with tests)."""
    weights = {
        "offset_w": np.asarray(offset_w, np.float32),
        "offset_b": np.asarray(offset_b, np.float32),
        "deform_w": np.asarray(deform_w, np.float32),
        "deform_b": np.asarray(deform_b, np.float32),
    }
    nc = _build_program(weights, debug_skip=debug_skip)
    x = np.asarray(x, np.float32)
    in_maps = [{"xpad": _host_prep(x[b])} for b in range(x.shape[0])]
    return nc, in_maps


def _postprocess(out_maps):
    outs = []
    for om in out_maps:
        o = np.asarray(om["out_t"], np.float32)  # [HW, 64]
        outs.append(o.reshape(H, W, COUT).transpose(2, 0, 1))
    return np.stack(outs).astype(np.float32)


def kernel(x, offset_w, offset_b, deform_w, deform_b):
    from concourse import bass_utils

    nc, in_maps = build(x, offset_w, offset_b, deform_w, deform_b)
    res = bass_utils.run_bass_kernel_spmd(nc, in_maps,
                                          core_ids=list(range(len(in_maps))))
    return _postprocess(res.results)



# revision 29
# speedup vs baseline: 1.0452x; 1.0452x over previous
"""DeformableConv2d Trainium2 kernel (V2).

Data-parallel over batch: 8 samples -> 8 NeuronCores, one sample per core.

Key identity: bilinear sampling is linear in the image, so it commutes with
the channel contraction:
    out[o,q] = sum_k bilinear(G_k, py_k(q), px_k(q))[o] + bias[o]
with G_k = W_k @ x (a 1x1 conv per tap); the (ki,kj) tap displacement is
folded into the gather position.

V2 layout: the DRAM workspace holds 256-B *y-pair records*
    rec_k(y, x) = [G_k(y,x) 64ch, G_k(y+1,x) 64ch]   (bf16)
on a 120x120 zero-padded grid (GPAD=4 absorbs offsets; zero pad == OOB-zero
semantics). One dma_gather element (elem_size 512B, elem_step 256B) at
idx = y0*120+x0 fetches records (y0,x0),(y0,x0+1) = ALL FOUR bilinear
corners [g00,g10,g01,g11] -- half the descriptors/instructions of the
2-gather x-pair scheme. The y-pair comes from a second, row-shifted PE
matmul (PE has headroom), not extra copies.

Combine: per (tap, half): acc += sum_c w_c * g_c with the 4 corner weights
precomputed per pixel and stored as *duplicated bf16 pairs* [w,w] so the
weight operand's innermost AP dim is step-1/count-2 -> DVE 2x_1P mode for
the big multiply (a 0-stride innermost broadcast would force 1x).

Device pipeline per core:
  A) offset conv on PE (bf16, 9 accumulating taps over padded x).
  B) index math on DVE: floor/frac; idx = y0*120+x0 (int16), replicated to
     128 partitions for the gather's 16-wrapped SWDGE layout; w4dup pair
     weights.
  C) G staging in 3 tap-groups of 3 (so gathers of group g overlap staging
     of group g+1): per grid column, two row-shifted matmuls into a
     bank-aligned [113, 2, 512] PSUM tile; one ScalarE drain interleaves
     [shift,k,o] into SBUF; per (28-column group, tap) SWDGE DMAs write
     y-pair records (small concurrent DMAs overlap on the rings, and
     per-tap granularity lets gather(k) wait only on tap k's slices);
     3 bulk DMAs zero-fill the pad bands.
  D) dma_gather (GPSIMD SWDGE) DRAM -> SBUF, 512-B elements, pixel-major
     output [q%128, q//128, 256]. Each (tap, half) is split into 2 chunk
     gathers so a tap's 4 chunks land on all 4 SWDGE queues -> all 4 Q7
     core pairs run descriptor generation concurrently (desc-gen is the
     dominant serial cost of dma_gather).
  E) DVE: one 2x-mode multiply by w4dup + 4 accumulate-adds per (tap,half).
"""

import numpy as np

K = 3
B, CIN, COUT, H, W = 8, 64, 64, 112, 112
HW = H * W                 # 12544
NPAD = H + 2               # 114 (conv pad=1)
GPAD = 4                   # padding of the gather grid
GP = H + 2 * GPAD          # 120
GROWS = GP * GP            # 14400 records in the gather grid
NQB = HW // 128            # 98 q-blocks of 128 pixels
NPLANE = 2 * K * K         # 18 offset planes (9 dy then 9 dx)
HALF = HW // 2             # 6272 pixels per gather half
HSLOT = HALF // 128        # 49
XG = 28                    # grid columns per staging DMA group
NXG = H // XG              # 4 groups (112 staged columns)
YR = H + 1                 # 113 staged record rows (y = 3..115)
REC = 2 * COUT             # 128 elems = 256 B per record

_CACHE = {}


def _build_program(weights, debug_skip=()):
    import ml_dtypes
    import concourse.bass as bass
    import concourse.bacc as bacc
    import concourse.mybir as mybir
    from concourse.tile import TileContext

    dt = mybir.dt
    ALU = mybir.AluOpType
    AP = bass.AP
    bf = ml_dtypes.bfloat16

    nc = bacc.Bacc("TRN2", target_bir_lowering=False, debug=False,
                   num_swdge_queues=4)

    offset_w = weights["offset_w"]  # [18, 64, 3, 3]
    offset_b = weights["offset_b"]  # [18]
    deform_w = weights["deform_w"]  # [64, 64, 3, 3]
    deform_b = weights["deform_b"]  # [64]

    # ---- host-side constants (baked into the NEFF) ----
    # offset conv lhsT per tap: [64, 18]; output plane j<9 = dy of tap j,
    # j>=9 = dx of tap j-9.
    woff = np.zeros((9, CIN, NPLANE), np.float32)
    for k in range(9):
        ki, kj = k // 3, k % 3
        for j in range(9):
            woff[k, :, j] = offset_w[2 * j, :, ki, kj]
            woff[k, :, 9 + j] = offset_w[2 * j + 1, :, ki, kj]

    # G conv rhs: wg[c, k*64+o] = deform_w[o, c, ki, kj]
    wg = np.zeros((CIN, 9 * COUT), np.float32)
    for k in range(9):
        ki, kj = k // 3, k % 3
        wg[:, k * 64:(k + 1) * 64] = deform_w[:, :, ki, kj].T

    # base grid [128, 18, 98] fp32 (+GPAD baked in, offset_b folded in)
    qs = np.arange(HW)
    ho, wo = qs // W, qs % W
    base = np.zeros((NPLANE, HW), np.float32)
    for k in range(9):
        ki, kj = k // 3, k % 3
        base[k] = ho + ki - 1 + GPAD + float(offset_b[2 * k])
        base[9 + k] = wo + kj - 1 + GPAD + float(offset_b[2 * k + 1])
    base_re = np.ascontiguousarray(
        base.reshape(NPLANE, NQB, 128).transpose(2, 0, 1))  # [128, 18, 98]

    bias_rep = np.broadcast_to(
        deform_b.astype(np.float32)[None, :], (128, COUT))

    woff_c = nc.inline_tensor(
        np.ascontiguousarray(woff.transpose(1, 0, 2)).reshape(
            CIN, 9 * NPLANE).astype(bf), name="woffc")
    wg_c = nc.inline_tensor(wg.astype(bf), name="wgc")
    base_c = nc.inline_tensor(
        base_re.reshape(128, NPLANE * NQB), name="basec")
    bias_c = nc.inline_tensor(np.ascontiguousarray(bias_rep).astype(bf),
                              name="biasc")
    ident_c = nc.inline_tensor(np.eye(NPLANE, dtype=np.float32).astype(bf),
                               name="ident")
    sel = np.zeros((8, 128, 16), np.float32)
    for s in range(8):
        for t in range(16):
            sel[s, 16 * s + t, t] = 1.0
    sel_c = nc.inline_tensor(
        np.ascontiguousarray(sel.transpose(1, 0, 2)).reshape(
            128, 8 * 16).astype(bf), name="selc")

    xpad = nc.dram_tensor("xpad", [CIN, NPAD * NPAD], dt.bfloat16,
                          kind="ExternalInput")
    out_t = nc.dram_tensor("out_t", [HW, COUT], dt.bfloat16,
                           kind="ExternalOutput")
    # +1 record per tap so a gather element at the last record stays within
    # the tap plane (idx 14399 reads records 14399,14400).
    gws = nc.dram_tensor("gws", [9, GROWS + 1, REC], dt.bfloat16,
                         kind="Internal")
    TAP = (GROWS + 1) * REC  # elem stride between tap planes

    with TileContext(nc) as tc:
        with (
            tc.tile_pool(name="const", bufs=1) as constp,
            tc.tile_pool(name="xsb", bufs=1) as xsbp,
            tc.tile_pool(name="persist", bufs=1) as perp,
            tc.tile_pool(name="accp", bufs=1) as accp,
            tc.tile_pool(name="zz", bufs=1) as zzp,
        ):
            # ---- constants + input ----
            woff_sb = constp.tile([CIN, 9, NPLANE], dt.bfloat16)
            nc.sync.dma_start(
                out=woff_sb[:],
                in_=woff_c[:].rearrange("c (k m) -> c k m", k=9))
            wg_sb = constp.tile([CIN, 9 * COUT], dt.bfloat16)
            nc.sync.dma_start(out=wg_sb[:], in_=wg_c[:])
            base_sb = constp.tile([128, NPLANE, NQB], dt.float32)
            nc.sync.dma_start(
                out=base_sb[:],
                in_=base_c[:].rearrange("p (a b) -> p a b", a=NPLANE))
            bias_sb = constp.tile([128, COUT], dt.bfloat16)
            nc.sync.dma_start(out=bias_sb[:], in_=bias_c[:])
            ident_sb = constp.tile([NPLANE, NPLANE], dt.bfloat16)
            nc.sync.dma_start(out=ident_sb[:], in_=ident_c[:])
            sel_sb = constp.tile([128, 8, 16], dt.bfloat16)
            nc.sync.dma_start(
                out=sel_sb[:], in_=sel_c[:].rearrange("p (s t) -> p s t", s=8))

            x_sb = xsbp.tile([CIN, NPAD, NPAD], dt.bfloat16)
            nc.sync.dma_start(
                out=x_sb[:], in_=xpad[:].rearrange("c (a b) -> c a b", a=NPAD))

            # ---- zero-fill of the record-grid pad bands ----
            # Content lives at record rows 3..115, cols 4..115. Zero bands
            # (record units, row-major y*120+x):
            #   top:    records 0..363        (rows 0-2 + row 3 cols 0-3)
            #   mid:    per row y in 3..114: the 8 contiguous records
            #           (y,116)..(y+1,3) straddling the row boundary
            #   bottom: records 13916..14400  ((115,116) .. the pad record)
            zz = zzp.tile([128, 1024], dt.bfloat16)
            nc.vector.memset(zz[:], 0)
            if "gstage" not in debug_skip:
                zzap = zz[:]
                ZSP = zzap.ap[0][0]
                for k in range(9):
                    gk = gws[k, :, :]
                    dst = AP(gk.tensor, gk.offset,
                             [[364, 128], [1, 364]])
                    src = AP(zzap.tensor, zzap.offset,
                             [[ZSP, 128], [1, 364]])
                    nc.sync.dma_start(out=dst, in_=src)
                    dst = AP(gk.tensor, gk.offset + (3 * GP + 116) * REC,
                             [[GP * REC, 112], [1, 1024]])
                    src = AP(zzap.tensor, zzap.offset,
                             [[ZSP, 112], [1, 1024]])
                    nc.sync.dma_start(out=dst, in_=src)
                    dst = AP(gk.tensor, gk.offset + (115 * GP + 116) * REC,
                             [[485, 128], [1, 485]])
                    src = AP(zzap.tensor, zzap.offset,
                             [[ZSP, 128], [1, 485]])
                    nc.sync.dma_start(out=dst, in_=src)

            # ---- A) offset conv + B) index math (scoped pools) ----
            ab = tc.tile_pool(name="idxm", bufs=1)
            idxp = ab.__enter__()
            psA_cm = tc.tile_pool(name="psA", bufs=2, space="PSUM")
            psA = psA_cm.__enter__()
            psT_cm = tc.tile_pool(name="psT", bufs=2, space="PSUM", side="right")
            psT = psT_cm.__enter__()
            off_sb = idxp.tile([NPLANE, HW], dt.bfloat16)
            RB = 4
            for blk in range(H // RB):
                ps = psA.tile([NPLANE, RB * W], dt.float32)
                for k in range(9):
                    ki, kj = k // 3, k % 3
                    rhs = x_sb[:, blk * RB + ki: blk * RB + ki + RB,
                               kj: kj + W]
                    nc.tensor.matmul(ps[:], woff_sb[:, k, :], rhs,
                                     start=(k == 0), stop=(k == 8))
                nc.scalar.copy(
                    out=off_sb[:, blk * RB * W:(blk + 1) * RB * W], in_=ps[:])
            psA_cm.__exit__(None, None, None)

            # rearrange [18, HW] -> [128, 18, 98] via PE transposes,
            # 8 transposes packed per PSUM tile -> 1 drain per 8 blocks.
            off_re = idxp.tile([128, NPLANE, NQB], dt.float32)
            qb = 0
            while qb < NQB:
                nn = min(8, NQB - qb)
                pst = psT.tile([128, 8, 32], dt.bfloat16, tag="pstT")
                for j in range(nn):
                    nc.tensor.transpose(
                        pst[:, j, 0:NPLANE],
                        off_sb[:, (qb + j) * 128:(qb + j + 1) * 128],
                        ident_sb[:])
                orap = off_re[:]
                OSP = orap.ap[0][0]
                prap = pst[:]
                PSP = prap.ap[0][0]
                dstr = AP(orap.tensor, orap.offset + qb,
                          [[OSP, 128], [NQB, NPLANE], [1, nn]])
                srcr = AP(prap.tensor, prap.offset,
                          [[PSP, 128], [1, NPLANE], [32, nn]])
                nc.scalar.copy(out=dstr, in_=srcr)
                qb += nn

            # ---- B) index math ----
            pyx = idxp.tile([128, NPLANE, NQB], dt.float32)
            nc.vector.tensor_add(out=pyx[:], in0=off_re[:], in1=base_sb[:])
            # floor(pyx) for pyx>=0: round-to-int via +-2^23, then
            # subtract 1 where rounding went up.
            M23 = 8388608.0
            rnd = idxp.tile([128, NPLANE, NQB], dt.float32)
            nc.vector.tensor_scalar(rnd[:], pyx[:], M23, M23, ALU.add,
                                    ALU.subtract)
            dgt = idxp.tile([128, NPLANE, NQB], dt.float32)
            nc.vector.tensor_tensor(out=dgt[:], in0=rnd[:], in1=pyx[:],
                                    op=ALU.is_gt)
            flr = rnd
            nc.vector.tensor_sub(out=flr[:], in0=rnd[:], in1=dgt[:])
            frac = dgt
            nc.vector.tensor_sub(out=frac[:], in0=pyx[:], in1=flr[:])
            # fractional weights as duplicated bf16 pairs [f, f] so the
            # combine multiply keeps DVE 2x_1P (innermost step-1 pair).
            fdup = idxp.tile([128, NPLANE, NQB, 2], dt.bfloat16)
            nc.vector.tensor_copy(
                out=fdup[:],
                in_=frac[:, :, :, None].to_broadcast((128, NPLANE, NQB, 2)))
            gdup = idxp.tile([128, NPLANE, NQB, 2], dt.bfloat16)
            nc.vector.tensor_scalar(gdup[:], fdup[:], -1.0, 1.0, ALU.mult,
                                    ALU.add)
            # 4 corner weights per (tap, pixel), pair-duplicated; corner
            # order matches the gathered element [g00, g10, g01, g11]:
            #   c0=wy0*wx0  c1=wy1*wx0  c2=wy0*wx1  c3=wy1*wx1
            w4dup = perp.tile([128, 9, 4, NQB, 2], dt.bfloat16)
            for c, (yt, xt) in enumerate(((0, 0), (1, 0), (0, 1), (1, 1))):
                wy = (gdup, fdup)[yt]
                wx = (gdup, fdup)[xt]
                nc.vector.tensor_mul(out=w4dup[:, :, c],
                                     in0=wy[:, 0:9], in1=wx[:, 9:18])
            abq = tc.tile_pool(name="idxb", bufs=1, side="right")
            idxq = abq.__enter__()
            flr_bf = idxq.tile([128, NPLANE, NQB], dt.bfloat16)
            nc.vector.tensor_copy(out=flr_bf[:], in_=flr[:])
            ab.__exit__(None, None, None)



            # ---- idx selection path first (PE/ScalarE ahead of staging) --
            # floor values are integers <= ~121 -> exact in bf16; fold
            # partition axis q_lo=(16s+t) into [16(t), ..., 8(s)] via 8
            # selection matmuls so the gather's 16-wrapped index layout is
            # reachable without per-element DMA. flrs is s-major so each
            # PSUM drain is contiguous; the idx combine below reorders to
            # the s-minor layout the gather consumes via its APs.
            flrs = idxq.tile([16, 8, NPLANE, 2, HSLOT], dt.bfloat16)
            for s in range(8):
                for (pl0, PL) in ((0, 5), (5, 4), (9, 5), (14, 4)):
                    pst2 = psT.tile([16, 5 * NQB], dt.float32, tag="pst2")
                    nc.tensor.matmul(pst2[:, 0:PL * NQB], sel_sb[:, s, :],
                                     flr_bf[:, pl0:pl0 + PL, :],
                                     start=True, stop=True)
                    nc.scalar.copy(
                        out=flrs[:, s, pl0:pl0 + PL, :, :],
                        in_=pst2[:, 0:PL * NQB].rearrange(
                            "p (a h q) -> p a h q", a=PL, h=2))
            # idx = yfloor*120 + xfloor (fp32 internal -> exact int16),
            # with the s-major -> s-minor reorder folded into the APs.
            idx_w16 = idxq.tile([16, 9, 2, HSLOT, 8], dt.int16)
            fap = flrs[:]
            FSP = fap.ap[0][0]
            iap = idx_w16[:]
            ISP2 = iap.ap[0][0]
            SSTRIDE = NPLANE * 2 * HSLOT
            for s in range(8):
                in_y = AP(fap.tensor, fap.offset + s * SSTRIDE,
                          [[FSP, 16], [2 * HSLOT, 9], [1, 2 * HSLOT]])
                in_x = AP(fap.tensor, fap.offset + s * SSTRIDE
                          + 9 * 2 * HSLOT,
                          [[FSP, 16], [2 * HSLOT, 9], [1, 2 * HSLOT]])
                out_i = AP(iap.tensor, iap.offset + s,
                           [[ISP2, 16], [2 * HSLOT * 8, 9], [8, 2 * HSLOT]])
                nc.vector.scalar_tensor_tensor(
                    out=out_i, in0=in_y, scalar=float(GP),
                    in1=in_x, op0=ALU.mult, op1=ALU.add)
            # replicate partitions 0:16 -> all 128 (idx_w persists)
            idx_w = perp.tile([128, 9, 2, 8 * HSLOT], dt.int16)
            i16 = idx_w16[:]
            wap = idx_w[:]
            WSPAN = wap.ap[0][0]
            ISPAN = i16.ap[0][0]
            rep_src = AP(i16.tensor, i16.offset,
                         [[ISPAN, 16], [1, ISPAN]])
            for g in range(8):
                rep_dst = AP(wap.tensor, wap.offset + 16 * g * WSPAN,
                             [[WSPAN, 16], [1, WSPAN]])
                nc.gpsimd.dma_start(out=rep_dst, in_=rep_src)

            psT_cm.__exit__(None, None, None)
            abq.__exit__(None, None, None)

            # ---- C) G staging ----
            psG_cm = tc.tile_pool(name="psG", bufs=4, space="PSUM")
            psG = psG_cm.__enter__()
            gsb_cm = tc.tile_pool(name="gsb", bufs=3)
            gsbp = gsb_cm.__enter__()

            def stage_group(kl, kh):
                # stage y-pair records for taps kl..kh-1 (all columns).
                NK = kh - kl
                for xg in range(NXG):
                    # k-major so the record DMA's inner run (per y,k) is
                    # the contiguous XG*REC block
                    gsb = gsbp.tile([YR, 3, XG, 2, COUT], dt.bfloat16)
                    for s in range(XG):
                        gx = 4 + xg * XG + s
                        psa = psG.tile([YR, 2, 512], dt.float32,
                                       tag="psa")
                        for sh in range(2):
                            lhsT = x_sb[:, sh:sh + YR, gx - 3]
                            nc.tensor.matmul(
                                psa[:, sh, 0:NK * COUT], lhsT,
                                wg_sb[:, kl * COUT:kh * COUT],
                                start=True, stop=True)
                        # drain [y, sh, k, o] -> gsb[y, k, s, sh, o]
                        ga = gsb[:]
                        GS = ga.ap[0][0]
                        pa = psa[:]
                        PS = pa.ap[0][0]
                        dsta = AP(ga.tensor,
                                  ga.offset + s * (2 * COUT),
                                  [[GS, YR], [COUT, 2],
                                   [XG * 2 * COUT, NK], [1, COUT]])
                        srca = AP(pa.tensor, pa.offset,
                                  [[PS, YR], [512, 2], [COUT, NK],
                                   [1, COUT]])
                        nc.scalar.copy(out=dsta, in_=srca)
                    # per-(xg, tap, y-half) record DMAs: one SWDGE DMA of
                    # this shape is serviced by ~one SDMA engine (~27 GB/s
                    # = 7KB descriptors at ~265ns each); aggregate rate
                    # scales with the number of DMAs in flight, so split
                    # finely. Per-tap granularity also lets gather(k) wait
                    # only on tap k's slices instead of the whole group.
                    ga = gsb[:]
                    GS = ga.ap[0][0]
                    for k in range(kl, kh):
                        gk = gws[k, :, :]
                        for y0, y1 in ((0, 57), (57, YR)):
                            dst = AP(gk.tensor,
                                     gk.offset
                                     + ((3 + y0) * GP + 4 + xg * XG) * REC,
                                     [[GP * REC, y1 - y0], [1, XG * REC]])
                            src = AP(ga.tensor,
                                     ga.offset + y0 * GS
                                     + (k - kl) * (XG * 2 * COUT),
                                     [[GS, y1 - y0], [1, XG * 2 * COUT]])
                            nc.gpsimd.dma_start(out=dst, in_=src)


            # ---- D/E) gather + combine, tap-pipelined ----
            gat_cm = tc.tile_pool(name="gat", bufs=2)
            gatp = gat_cm.__enter__()
            acc = accp.tile([128, NQB, COUT], dt.bfloat16)
            nc.vector.tensor_copy(
                out=acc[:],
                in_=bias_sb[:, None, :].to_broadcast((128, NQB, COUT)))

            def gather_combine(k):
                for h in range(2):
                    gg = gatp.tile([128, HSLOT, 4 * COUT], dt.bfloat16,
                                   tag="gg")
                    gk = gws[k, :, :]
                    src0 = AP(gk.tensor, gk.offset,
                              [[REC, GROWS], [1, 2 * REC]])
                    # split each half across 2 SWDGE queues (so each tap's
                    # 4 chunks land on all 4 queues = all 4 Q7 core pairs
                    # desc-gen concurrently). Chunk boundary at a slot
                    # multiple (f multiple of 8) keeps out slots aligned.
                    for c, (s0, s1) in enumerate(((0, 25), (25, HSLOT))):
                        nidx = (s1 - s0) * 128
                        idxs = idx_w[:, k, h, s0 * 8:s1 * 8]
                        nc.gpsimd.dma_gather(
                            out_ap=gg[:, s0:s1, :], in_ap=src0, idxs_ap=idxs,
                            num_idxs=nidx, num_idxs_reg=nidx,
                            elem_size=2 * REC, elem_step=REC,
                            single_packet=False, queue_num=(2 * h + c) % 4)

                    # per-corner m_c = g_c * w_c (in-place, 2x_1P via the
                    # step-1 pair weights; ISA caps TT at 3 free dims so
                    # one op per corner), then acc += m_c.
                    wt = w4dup[:]
                    WSP = wt.ap[0][0]
                    ggv = gg[:].rearrange("p s (c u d) -> p s c u d",
                                          c=4, d=2)
                    a = acc[:, h * HSLOT:(h + 1) * HSLOT, :]
                    gc = gg[:].rearrange("p s (c o) -> p s c o", c=4)
                    for c in range(4):
                        woff0 = (wt.offset + k * (4 * NQB * 2)
                                 + c * (NQB * 2) + h * (HSLOT * 2))
                        win = AP(wt.tensor, woff0,
                                 [[WSP, 128], [2, HSLOT],
                                  [0, COUT // 2], [1, 2]])
                        gcv = ggv[:, :, c]
                        nc.vector.tensor_mul(out=gcv, in0=gcv, in1=win)
                        nc.vector.tensor_add(out=a, in0=a,
                                             in1=gc[:, :, c, :])

            GROUPS = ((0, 3), (3, 6), (6, 9))
            if "gstage" not in debug_skip:
                for gi, (kl, kh) in enumerate(GROUPS):
                    stage_group(kl, kh)
                    if "gather" not in debug_skip:
                        for k in range(kl, kh):
                            gather_combine(k)

            gat_cm.__exit__(None, None, None)
            gsb_cm.__exit__(None, None, None)
            psG_cm.__exit__(None, None, None)

            # out: [128, 98, 64] -> DRAM [12544, 64] with q = qblk*128+q_lo
            oap2 = out_t[:]
            dst = AP(oap2.tensor, 0, [[COUT, 128], [128 * COUT, NQB],
                                      [1, COUT]])
            nc.sync.dma_start(out=dst, in_=acc[:])

    nc.compile()
    return nc


def _host_prep(x):
    """Per-core input prep: pad=1 + bf16 + channel-major."""
    import ml_dtypes
    xp = np.zeros((CIN, NPAD, NPAD), np.float32)
    xp[:, 1:113, 1:113] = x
    return np.ascontiguousarray(
        xp.reshape(CIN, NPAD * NPAD)).astype(ml_dtypes.bfloat16)


def build(x, offset_w, offset_b, deform_w, deform_b, debug_skip=()):
    """Build the Bass program + per-core input maps (shared with tests)."""
    weights = {
        "offset_w": np.asarray(offset_w, np.float32),
        "offset_b": np.asarray(offset_b, np.float32),
        "deform_w": np.asarray(deform_w, np.float32),
        "deform_b": np.asarray(deform_b, np.float32),
    }
    nc = _build_program(weights, debug_skip=debug_skip)
    x = np.asarray(x, np.float32)
    in_maps = [{"xpad": _host_prep(x[b])} for b in range(x.shape[0])]
    return nc, in_maps


def _postprocess(out_maps):
    outs = []
    for om in out_maps:
        o = np.asarray(om["out_t"], np.float32)  # [HW, 64]
        outs.append(o.reshape(H, W, COUT).transpose(2, 0, 1))
    return np.stack(outs).astype(np.float32)


def kernel(x, offset_w, offset_b, deform_w, deform_b):
    from concourse import bass_utils

    nc, in_maps = build(x, offset_w, offset_b, deform_w, deform_b)
    res = bass_utils.run_bass_kernel_spmd(nc, in_maps,
                                          core_ids=list(range(len(in_maps))))
    return _postprocess(res.results)

